# revision 21
# baseline (speedup 1.0000x reference)
"""Trainium2 Bass kernel for nn_ExistLCross (masked weighted -log loss).

reference:
    mask = (label == 1)
    per_elem = -log(pred + 0.01) * existmap * mask
    loss = einsum('nchw,c->', per_elem, Wl) / sum(label_sum)

Sharding: data-parallel over batch N=8 -> one batch item per NeuronCore.
Each core processes its [C=16, 512, 512] shard as 16 planes of [128, 2048]
(the last class in 4 quarter-plane chunks to shorten the post-DMA tail):
    ACT : logp = Ln(pred + 0.01)            (bias folded into activation)
    DVE : prod = logp * existmap            (tensor_tensor)
    DVE : scr  = (label == 1) * prod        (scalar_tensor_tensor)
          accum_out -> acc[:, col]  (per-partition sums, one col per chunk)
Each core DMAs acc [128, 19] back; the host applies the per-class weights
-Wl/sum(label_sum) and adds up the 8 per-core partials (the all-reduce).
"""

from contextlib import ExitStack

import numpy as np

import concourse.bacc as bacc
import concourse.bass as bass
import concourse.tile as tile
from concourse import bass_utils, mybir

N, C, H, W = 8, 16, 512, 512
P = 128
FREE = (H * W) // P  # 2048
EPS = 0.01
N_CORES = 8
NACC = C + 3  # last class spread over 4 accumulator columns

_nc_cache = []


def _build_nc() -> bass.Bass:
    nc = bacc.Bacc("TRN2", target_bir_lowering=False, debug=False,
                   num_devices=N_CORES)

    pred_d = nc.dram_tensor("pred", [C, P, FREE], mybir.dt.float32,
                            kind="ExternalInput").ap()
    lab_d = nc.dram_tensor("label", [C, P, FREE], mybir.dt.int32,
                           kind="ExternalInput").ap()
    em_d = nc.dram_tensor("existmap", [C, P, FREE], mybir.dt.float32,
                          kind="ExternalInput").ap()
    out_d = nc.dram_tensor("out", [P, NACC], mybir.dt.float32,
                           kind="ExternalOutput").ap()

    with tile.TileContext(nc) as tc, ExitStack() as ctx:
        ins = ctx.enter_context(tc.tile_pool(name="ins", bufs=4))
        work = ctx.enter_context(tc.tile_pool(name="work", bufs=2))
        singles = ctx.enter_context(tc.tile_pool(name="singles", bufs=1))

        acc = singles.tile([P, NACC], mybir.dt.float32)
        ones = singles.tile([P, 1], mybir.dt.float32)
        eps_t = singles.tile([P, 1], mybir.dt.float32)
        nc.vector.memset(ones, 1.0)
        # eps = ones*0 + EPS, produced on ACT so every Ln below depends on
        # it same-engine (no cross-engine wait, no pre-Tile barrier).
        nc.scalar.activation(eps_t, ones, mybir.ActivationFunctionType.Copy,
                             bias=EPS, scale=0.0)

        def compute(pred_v, lab_v, em_v, fr, acc_col):
            """log/mask chain on [P, fr] SBUF views, accumulating the
            per-partition sums into acc[:, acc_col]."""
            logp = work.tile([P, fr], mybir.dt.float32, tag="logp")
            nc.scalar.activation(logp, pred_v,
                                 mybir.ActivationFunctionType.Ln, bias=eps_t)

            prod = work.tile([P, fr], mybir.dt.float32, tag="prod")
            nc.vector.tensor_mul(prod, logp, em_v)

            scr = work.tile([P, fr], mybir.dt.float32, tag="scr")
            nc.vector.scalar_tensor_tensor(
                out=scr, in0=lab_v, scalar=1.0, in1=prod,
                op0=mybir.AluOpType.is_equal, op1=mybir.AluOpType.mult,
                accum_out=acc[:, acc_col:acc_col + 1],
            )

        def load(pred_ap, lab_ap, em_ap, fr):
            """One [P, fr] contiguous block of each input on the SP ring."""
            pred_t = ins.tile([P, fr], mybir.dt.float32, tag="pred")
            lab_t = ins.tile([P, fr], mybir.dt.int32, tag="lab")
            em_t = ins.tile([P, fr], mybir.dt.float32, tag="em")
            nc.sync.dma_start(out=pred_t, in_=pred_ap)
            nc.sync.dma_start(out=lab_t, in_=lab_ap)
            nc.sync.dma_start(out=em_t, in_=em_ap)
            return pred_t, lab_t, em_t

        # prime all 8 DMAHW lanes with 64 B reads so the first big
        # transfers don't pay per-queue first-use setup serially
        warm = singles.tile([1, 8, 16], mybir.dt.float32)
        for w in range(8):
            nc.sync.dma_start(out=warm[:, w], in_=pred_d[0, 0:1, 0:16])

        for c in range(C - 1):
            tiles = load(pred_d[c], lab_d[c], em_d[c], FREE)
            compute(*tiles, FREE, c)

        # last class: 4 quarter chunks to shorten the post-DMA tail
        c = C - 1
        Q = FREE // 4
        for q in range(4):
            sl = slice(q * Q, (q + 1) * Q)
            tiles = load(pred_d[c, :, sl], lab_d[c, :, sl],
                         em_d[c, :, sl], Q)
            compute(*tiles, Q, C - 1 + q)

        nc.sync.dma_start(out=out_d, in_=acc)

    nc.compile()
    return nc


def _get_nc() -> bass.Bass:
    if not _nc_cache:
        _nc_cache.append(_build_nc())
    return _nc_cache[0]


def _run(pred, label, Wl, label_sum, existmap, **spmd_kwargs):
    pred = np.ascontiguousarray(np.asarray(pred, dtype=np.float32))
    label = np.ascontiguousarray(np.asarray(label, dtype=np.int32))
    Wl = np.asarray(Wl, dtype=np.float32)
    label_sum = np.asarray(label_sum, dtype=np.float32)
    existmap = np.ascontiguousarray(np.asarray(existmap, dtype=np.float32))

    denom = np.sum(label_sum, dtype=np.float32)
    wl_scaled = (-Wl / denom).astype(np.float32)
    # last class occupies 4 accumulator columns (quarter-plane chunks)
    wl_ext = np.concatenate([wl_scaled, np.repeat(wl_scaled[-1:], 3)])

    in_maps = []
    for i in range(N_CORES):
        in_maps.append({
            "pred": pred[i].reshape(C, P, FREE),
            "label": label[i].reshape(C, P, FREE),
            "existmap": existmap[i].reshape(C, P, FREE),
        })

    nc = _get_nc()
    res = bass_utils.run_bass_kernel_spmd(
        nc, in_maps, core_ids=list(range(N_CORES)), **spmd_kwargs)

    total = np.float32(0.0)
    for r in res.results:
        # [P, NACC] partition sums -> weighted scalar (f32 like reference)
        per_class = r["out"].sum(axis=0, dtype=np.float32)
        total += np.float32((per_class * wl_ext).sum(dtype=np.float32))
    return np.array(total, dtype=np.float32), res


def kernel(pred, label, Wl, label_sum, existmap):
    out, _ = _run(pred, label, Wl, label_sum, existmap)
    return out


# revision 22
# speedup vs baseline: 1.2782x; 1.2782x over previous
"""Trainium2 Bass kernel for nn_ExistLCross (masked weighted -log loss).

reference:
    mask = (label == 1)
    per_elem = -log(pred + 0.01) * existmap * mask
    loss = einsum('nchw,c->', per_elem, Wl) / sum(label_sum)

Sharding: data-parallel over batch N=8 -> one batch item per NeuronCore.
Each core processes its [C=16, 512, 512] shard as 16 planes of [128, 2048]
(the last class in 4 quarter-plane chunks to shorten the post-DMA tail):
    ACT : logp = Ln(pred + 0.01)            (bias folded into activation)
    DVE : prod = logp * existmap            (tensor_tensor)
    DVE : scr  = (label == 1) * prod        (scalar_tensor_tensor)
          accum_out -> acc[:, col]  (per-partition sums, one col per chunk)
Each core DMAs acc [128, 19] back; the host applies the per-class weights
-Wl/sum(label_sum) and adds up the 8 per-core partials (the all-reduce).
"""

from contextlib import ExitStack

import numpy as np

import concourse.bacc as bacc
import concourse.bass as bass
import concourse.tile as tile
from concourse import bass_utils, mybir

N, C, H, W = 8, 16, 512, 512
P = 128
FREE = (H * W) // P  # 2048
EPS = 0.01
N_CORES = 8
NACC = C + 3  # last class spread over 4 accumulator columns

_nc_cache = []


def _build_nc() -> bass.Bass:
    nc = bacc.Bacc("TRN2", target_bir_lowering=False, debug=False,
                   num_devices=N_CORES)

    pred_d = nc.dram_tensor("pred", [C, P, FREE], mybir.dt.float32,
                            kind="ExternalInput").ap()
    lab_d = nc.dram_tensor("label", [C, P, FREE], mybir.dt.int32,
                           kind="ExternalInput").ap()
    em_d = nc.dram_tensor("existmap", [C, P, FREE], mybir.dt.float32,
                          kind="ExternalInput").ap()
    out_d = nc.dram_tensor("out", [P, NACC], mybir.dt.float32,
                           kind="ExternalOutput").ap()

    with tile.TileContext(nc) as tc, ExitStack() as ctx:
        ins = ctx.enter_context(tc.tile_pool(name="ins", bufs=4))
        work = ctx.enter_context(tc.tile_pool(name="work", bufs=2))
        singles = ctx.enter_context(tc.tile_pool(name="singles", bufs=1))

        acc = singles.tile([P, NACC], mybir.dt.float32)
        ones = singles.tile([P, 1], mybir.dt.float32)
        eps_t = singles.tile([P, 1], mybir.dt.float32)
        nc.vector.memset(ones, 1.0)
        # eps = ones*0 + EPS, produced on ACT so every Ln below depends on
        # it same-engine (no cross-engine wait, no pre-Tile barrier).
        nc.scalar.activation(eps_t, ones, mybir.ActivationFunctionType.Copy,
                             bias=EPS, scale=0.0)

        def compute(pred_v, lab_v, em_v, fr, acc_col):
            """log/mask chain on [P, fr] SBUF views, accumulating the
            per-partition sums into acc[:, acc_col]."""
            logp = work.tile([P, fr], mybir.dt.float32, tag="logp")
            nc.scalar.activation(logp, pred_v,
                                 mybir.ActivationFunctionType.Ln, bias=eps_t)

            prod = work.tile([P, fr], mybir.dt.float32, tag="prod")
            nc.vector.tensor_mul(prod, logp, em_v)

            scr = work.tile([P, fr], mybir.dt.float32, tag="scr")
            nc.vector.scalar_tensor_tensor(
                out=scr, in0=lab_v, scalar=1.0, in1=prod,
                op0=mybir.AluOpType.is_equal, op1=mybir.AluOpType.mult,
                accum_out=acc[:, acc_col:acc_col + 1],
            )

        def load(pred_ap, lab_ap, em_ap, fr):
            """One [P, fr] contiguous block of each input on the SP ring."""
            pred_t = ins.tile([P, fr], mybir.dt.float32, tag="pred")
            lab_t = ins.tile([P, fr], mybir.dt.int32, tag="lab")
            em_t = ins.tile([P, fr], mybir.dt.float32, tag="em")
            nc.sync.dma_start(out=pred_t, in_=pred_ap)
            nc.sync.dma_start(out=lab_t, in_=lab_ap)
            nc.sync.dma_start(out=em_t, in_=em_ap)
            return pred_t, lab_t, em_t

        for c in range(C - 1):
            tiles = load(pred_d[c], lab_d[c], em_d[c], FREE)
            compute(*tiles, FREE, c)

        # last class: 4 quarter chunks to shorten the post-DMA tail
        c = C - 1
        Q = FREE // 4
        for q in range(4):
            sl = slice(q * Q, (q + 1) * Q)
            tiles = load(pred_d[c, :, sl], lab_d[c, :, sl],
                         em_d[c, :, sl], Q)
            compute(*tiles, Q, C - 1 + q)

        nc.sync.dma_start(out=out_d, in_=acc)

    nc.compile()
    return nc


def _get_nc() -> bass.Bass:
    if not _nc_cache:
        _nc_cache.append(_build_nc())
    return _nc_cache[0]


def _run(pred, label, Wl, label_sum, existmap, **spmd_kwargs):
    pred = np.ascontiguousarray(np.asarray(pred, dtype=np.float32))
    label = np.ascontiguousarray(np.asarray(label, dtype=np.int32))
    Wl = np.asarray(Wl, dtype=np.float32)
    label_sum = np.asarray(label_sum, dtype=np.float32)
    existmap = np.ascontiguousarray(np.asarray(existmap, dtype=np.float32))

    denom = np.sum(label_sum, dtype=np.float32)
    wl_scaled = (-Wl / denom).astype(np.float32)
    # last class occupies 4 accumulator columns (quarter-plane chunks)
    wl_ext = np.concatenate([wl_scaled, np.repeat(wl_scaled[-1:], 3)])

    in_maps = []
    for i in range(N_CORES):
        in_maps.append({
            "pred": pred[i].reshape(C, P, FREE),
            "label": label[i].reshape(C, P, FREE),
            "existmap": existmap[i].reshape(C, P, FREE),
        })

    nc = _get_nc()
    res = bass_utils.run_bass_kernel_spmd(
        nc, in_maps, core_ids=list(range(N_CORES)), **spmd_kwargs)

    total = np.float32(0.0)
    for r in res.results:
        # [P, NACC] partition sums -> weighted scalar (f32 like reference)
        per_class = r["out"].sum(axis=0, dtype=np.float32)
        total += np.float32((per_class * wl_ext).sum(dtype=np.float32))
    return np.array(total, dtype=np.float32), res


def kernel(pred, label, Wl, label_sum, existmap):
    out, _ = _run(pred, label, Wl, label_sum, existmap)
    return out
